# revision 9
# baseline (speedup 1.0000x reference)
"""Trainium2 kernel for nn_ClusterBBoxes (NMS-style bbox clustering).

Strategy (row-sharded per spec hint):
  - 8 NeuronCores each compute a 1024-row block of the symmetric
    edge matrix  edge[i,j] = (IoU(i,j) > 0.1, i != j)  entirely on device
    (the O(N^2) compute that dominates the roofline).
  - The sequential single-pass union sweep (inherently a ~57K-edge
    dependent chain) and the tiny O(N) mask postprocess run on the host
    from the device-produced edge bits, faithfully replicating the
    reference semantics bit-exactly.

kernel(**inputs) takes FULL inputs and returns the FULL boolean mask.
Self-contained: no imports from the problem directory.
"""
import os
import numpy as np

N = 8192
BLK = 1024          # rows per core
P = 128
NT = BLK // P       # row tiles per core
IOU_THR = np.float32(0.1)

_compiled = None
last_exec_ns = None


def _build():
    import concourse.bass as bass
    import concourse.bacc as bacc
    import concourse.mybir as mybir
    from concourse.tile import TileContext

    nc = bacc.Bacc("TRN2", target_bir_lowering=False, debug=False)
    bb = nc.dram_tensor("bboxes", [N, 4], mybir.dt.float32, kind="ExternalInput")
    feat_d = nc.dram_tensor("feat_d", [6, N], mybir.dt.float32)  # x1,y1,x2,y2,area,iota
    edges = nc.dram_tensor("edges", [BLK, N], mybir.dt.uint8, kind="ExternalOutput")

    f32 = mybir.dt.float32
    Alu = mybir.AluOpType

    with TileContext(nc) as tc:
        cid = nc.partition_id()
        with tc.tile_pool(name="c", bufs=1) as cpool, tc.tile_pool(name="w", bufs=1) as wpool, tc.tile_pool(name="e", bufs=2) as epool:
            # ---- per-box features on [128, 64] node layout (node = p*64 + f)
            cx = cpool.tile([P, 64], f32)
            cy = cpool.tile([P, 64], f32)
            w_ = cpool.tile([P, 64], f32)
            h_ = cpool.tile([P, 64], f32)
            for k, t in enumerate((cx, cy, w_, h_)):
                nc.sync.dma_start(out=t[:], in_=bb[:, k:k + 1].rearrange("(p f) o -> p (f o)", p=P))
            x1 = cpool.tile([P, 64], f32)
            y1 = cpool.tile([P, 64], f32)
            x2 = cpool.tile([P, 64], f32)
            y2 = cpool.tile([P, 64], f32)
            area = cpool.tile([P, 64], f32)
            tmp = cpool.tile([P, 64], f32)
            # x1 = cx - 0.5*w ; x2 = cx + 0.5*w  (mirror reference op order)
            nc.vector.tensor_scalar(out=tmp[:], in0=w_[:], scalar1=0.5, scalar2=None, op0=Alu.mult)
            nc.vector.tensor_tensor(out=x1[:], in0=cx[:], in1=tmp[:], op=Alu.subtract)
            nc.vector.tensor_tensor(out=x2[:], in0=cx[:], in1=tmp[:], op=Alu.add)
            nc.vector.tensor_scalar(out=tmp[:], in0=h_[:], scalar1=0.5, scalar2=None, op0=Alu.mult)
            nc.vector.tensor_tensor(out=y1[:], in0=cy[:], in1=tmp[:], op=Alu.subtract)
            nc.vector.tensor_tensor(out=y2[:], in0=cy[:], in1=tmp[:], op=Alu.add)
            # area = (x2-x1)*(y2-y1)
            t2 = cpool.tile([P, 64], f32)
            nc.vector.tensor_tensor(out=tmp[:], in0=x2[:], in1=x1[:], op=Alu.subtract)
            nc.vector.tensor_tensor(out=t2[:], in0=y2[:], in1=y1[:], op=Alu.subtract)
            nc.vector.tensor_tensor(out=area[:], in0=tmp[:], in1=t2[:], op=Alu.mult)
            # stash features to DRAM [5, N] flat (node-major)
            for k, t in enumerate((x1, y1, x2, y2, area)):
                nc.sync.dma_start(out=feat_d[k:k + 1, :].rearrange("o (p f) -> p (o f)", p=P), in_=t[:])

            # ---- column index iota as f32, stored to DRAM row 5
            coli = cpool.tile([1, 2048], mybir.dt.int32, tag="coli")
            colif = cpool.tile([1, 2048], f32, tag="colif")
            for q in range(4):
                nc.gpsimd.iota(coli[:], pattern=[[1, 2048]], base=q * 2048, channel_multiplier=0)
                nc.vector.tensor_copy(out=colif[:], in_=coli[:])
                nc.sync.dma_start(out=feat_d[5:6, q * 2048:(q + 1) * 2048], in_=colif[:])

            base_reg = cid * BLK
            CH = 2048
            for q in range(4):
                # column-broadcast feature slices [1, CH]
                colf = []
                for k in range(6):
                    ct = wpool.tile([P, CH], f32, tag=f"col{k}")
                    nc.sync.dma_start(out=ct[0:1, :], in_=feat_d[k:k + 1, q * CH:(q + 1) * CH])
                    nc.gpsimd.partition_broadcast(ct[:], ct[0:1, :])
                    colf.append(ct)
                cx1b, cy1b, cx2b, cy2b, careab, colib = colf
                for t in range(NT):
                    start = base_reg + t * P
                    rx1 = wpool.tile([P, 1], f32, tag="rx1")
                    ry1 = wpool.tile([P, 1], f32, tag="ry1")
                    rx2 = wpool.tile([P, 1], f32, tag="rx2")
                    ry2 = wpool.tile([P, 1], f32, tag="ry2")
                    rar = wpool.tile([P, 1], f32, tag="rar")
                    ridx = wpool.tile([P, 1], f32, tag="ridx")
                    for k, rt in enumerate((rx1, ry1, rx2, ry2, rar, ridx)):
                        nc.sync.dma_start(out=rt[:], in_=feat_d[k, bass.ds(start, P), None])

                    e1 = epool.tile([P, CH], f32, tag="e1")
                    e2 = epool.tile([P, CH], f32, tag="e2")
                    e3 = epool.tile([P, CH], f32, tag="e3")
                    e4 = epool.tile([P, CH], f32, tag="e4")
                    e5 = epool.tile([P, CH], f32, tag="e5")
                    # x-chain on DVE
                    nc.vector.tensor_scalar(out=e1[:], in0=cx1b[:], scalar1=rx1[:], scalar2=None, op0=Alu.max)
                    nc.vector.scalar_tensor_tensor(out=e2[:], in0=cx2b[:], scalar=rx2[:], in1=e1[:], op0=Alu.min, op1=Alu.subtract)
                    nc.gpsimd.tensor_scalar(out=e2[:], in0=e2[:], scalar1=0.0, scalar2=None, op0=Alu.max)
                    # y-chain
                    nc.vector.tensor_scalar(out=e4[:], in0=cy1b[:], scalar1=ry1[:], scalar2=None, op0=Alu.max)
                    nc.vector.scalar_tensor_tensor(out=e3[:], in0=cy2b[:], scalar=ry2[:], in1=e4[:], op0=Alu.min, op1=Alu.subtract)
                    nc.gpsimd.tensor_scalar(out=e3[:], in0=e3[:], scalar1=0.0, scalar2=None, op0=Alu.max)
                    # union pre-add + diag stay on DVE (AP scalars unsupported on Pool)
                    nc.vector.tensor_scalar(out=e5[:], in0=careab[:], scalar1=rar[:], scalar2=None, op0=Alu.add)
                    nc.vector.tensor_scalar(out=e4[:], in0=colib[:], scalar1=ridx[:], scalar2=None, op0=Alu.not_equal)
                    # inter, union, edge on DVE
                    nc.gpsimd.tensor_tensor(out=e2[:], in0=e2[:], in1=e3[:], op=Alu.mult)
                    nc.vector.tensor_tensor(out=e1[:], in0=e5[:], in1=e2[:], op=Alu.subtract)
                    nc.vector.scalar_tensor_tensor(out=e1[:], in0=e1[:], scalar=float(IOU_THR), in1=e2[:], op0=Alu.mult, op1=Alu.is_lt)
                    eo = epool.tile([P, CH], mybir.dt.uint8, tag="eo")
                    nc.vector.tensor_tensor(out=eo[:], in0=e1[:], in1=e4[:], op=Alu.mult)
                    nc.sync.dma_start(out=edges[t * P:(t + 1) * P, q * CH:(q + 1) * CH], in_=eo[:])

    nc.compile()
    return nc


def _get_compiled():
    global _compiled
    if _compiled is None:
        _compiled = _build()
    return _compiled


def _host_sweep_and_mask(edge_sym: np.ndarray, conf: np.ndarray) -> np.ndarray:
    """Faithful replication of the reference's sequential sweep + mask build,
    operating on the symmetric edge matrix produced by the device."""
    upper = np.triu(edge_sym, 1).astype(bool)
    a = np.arange(N, dtype=np.int64)
    rows, cols = np.nonzero(upper)
    # uniform per-edge op in lexicographic order:  t = min(a[i],a[j]); a[i]=a[j]=t
    order_rows = rows  # already lex sorted by nonzero
    for i, j in zip(order_rows.tolist(), cols.tolist()):
        ai = a[i]; aj = a[j]
        t = ai if ai < aj else aj
        a[i] = t; a[j] = t
    labels = a
    conf = conf.astype(np.float32)
    cnt = np.zeros(N, np.int64)
    np.add.at(cnt, labels, 1)
    mc = np.full(N, -np.inf, np.float32)
    np.maximum.at(mc, labels, conf)
    cand_g = np.where(conf == mc[labels], np.arange(N), N)
    g = np.full(N, N, np.int64)
    np.minimum.at(g, labels, cand_g)
    gl = g[labels]
    lt = (np.arange(N) < gl).astype(np.int64)
    posr = np.zeros(N, np.int64)
    np.add.at(posr, labels, lt)
    mask = np.zeros(N, bool)
    mask |= (cnt[labels] == 1)
    multi = cnt >= 2
    mask[np.clip(posr[multi], 0, N - 1)] = True
    return mask


def kernel(bboxes_cxcywh: np.ndarray, conf: np.ndarray) -> np.ndarray:
    global last_exec_ns
    from concourse.bass_utils import run_bass_kernel_spmd

    nc = _get_compiled()
    bb = np.ascontiguousarray(bboxes_cxcywh, dtype=np.float32)
    in_maps = [{"bboxes": bb} for _ in range(8)]
    trace = bool(int(os.environ.get("KERNEL_TRACE", "0")))
    res = run_bass_kernel_spmd(nc, in_maps, list(range(8)), trace=trace)
    last_exec_ns = res.exec_time_ns
    edge_sym = np.concatenate([res.results[c]["edges"] for c in range(8)], axis=0)
    return _host_sweep_and_mask(edge_sym, np.asarray(conf))


# revision 10
# speedup vs baseline: 3.3679x; 3.3679x over previous
"""Trainium2 kernel for nn_ClusterBBoxes (NMS-style bbox clustering).

Strategy (row-sharded per spec hint):
  - 8 NeuronCores each compute a 1024-row block of the symmetric
    edge matrix  edge[i,j] = (IoU(i,j) > 0.1, i != j)  entirely on device
    (the O(N^2) compute that dominates the roofline).
  - The sequential single-pass union sweep (inherently a ~57K-edge
    dependent chain) and the tiny O(N) mask postprocess run on the host
    from the device-produced edge bits, faithfully replicating the
    reference semantics bit-exactly.

kernel(**inputs) takes FULL inputs and returns the FULL boolean mask.
Self-contained: no imports from the problem directory.
"""
import os
import numpy as np

N = 8192
BLK = 1024          # rows per core
P = 128
NT = BLK // P       # row tiles per core
IOU_THR = np.float32(0.1)

_compiled = None
last_exec_ns = None


def _build():
    import concourse.bass as bass
    import concourse.bacc as bacc
    import concourse.mybir as mybir
    from concourse.tile import TileContext

    nc = bacc.Bacc("TRN2", target_bir_lowering=False, debug=False)
    bb = nc.dram_tensor("bboxes", [N, 4], mybir.dt.float32, kind="ExternalInput")
    feat_d = nc.dram_tensor("feat_d", [6, N], mybir.dt.float32)  # x1,y1,x2,y2,area,iota
    edges = nc.dram_tensor("edges", [BLK, N], mybir.dt.uint8, kind="ExternalOutput")

    f32 = mybir.dt.float32
    Alu = mybir.AluOpType

    with TileContext(nc) as tc:
        cid = nc.partition_id()
        with tc.tile_pool(name="c", bufs=1) as cpool, tc.tile_pool(name="w", bufs=1) as wpool, tc.tile_pool(name="e", bufs=2) as epool:
            # ---- per-box features on [128, 64] node layout (node = p*64 + f)
            cx = cpool.tile([P, 64], f32)
            cy = cpool.tile([P, 64], f32)
            w_ = cpool.tile([P, 64], f32)
            h_ = cpool.tile([P, 64], f32)
            for k, t in enumerate((cx, cy, w_, h_)):
                nc.sync.dma_start(out=t[:], in_=bb[:, k:k + 1].rearrange("(p f) o -> p (f o)", p=P))
            x1 = cpool.tile([P, 64], f32)
            y1 = cpool.tile([P, 64], f32)
            x2 = cpool.tile([P, 64], f32)
            y2 = cpool.tile([P, 64], f32)
            area = cpool.tile([P, 64], f32)
            tmp = cpool.tile([P, 64], f32)
            # x1 = cx - 0.5*w ; x2 = cx + 0.5*w  (mirror reference op order)
            nc.vector.tensor_scalar(out=tmp[:], in0=w_[:], scalar1=0.5, scalar2=None, op0=Alu.mult)
            nc.vector.tensor_tensor(out=x1[:], in0=cx[:], in1=tmp[:], op=Alu.subtract)
            nc.vector.tensor_tensor(out=x2[:], in0=cx[:], in1=tmp[:], op=Alu.add)
            nc.vector.tensor_scalar(out=tmp[:], in0=h_[:], scalar1=0.5, scalar2=None, op0=Alu.mult)
            nc.vector.tensor_tensor(out=y1[:], in0=cy[:], in1=tmp[:], op=Alu.subtract)
            nc.vector.tensor_tensor(out=y2[:], in0=cy[:], in1=tmp[:], op=Alu.add)
            # area = (x2-x1)*(y2-y1)
            t2 = cpool.tile([P, 64], f32)
            nc.vector.tensor_tensor(out=tmp[:], in0=x2[:], in1=x1[:], op=Alu.subtract)
            nc.vector.tensor_tensor(out=t2[:], in0=y2[:], in1=y1[:], op=Alu.subtract)
            nc.vector.tensor_tensor(out=area[:], in0=tmp[:], in1=t2[:], op=Alu.mult)
            # stash features to DRAM [5, N] flat (node-major)
            for k, t in enumerate((x1, y1, x2, y2, area)):
                nc.sync.dma_start(out=feat_d[k:k + 1, :].rearrange("o (p f) -> p (o f)", p=P), in_=t[:])

            # ---- column index iota as f32, stored to DRAM row 5
            coli = cpool.tile([1, 2048], mybir.dt.int32, tag="coli")
            colif = cpool.tile([1, 2048], f32, tag="colif")
            for q in range(4):
                nc.gpsimd.iota(coli[:], pattern=[[1, 2048]], base=q * 2048, channel_multiplier=0)
                nc.vector.tensor_copy(out=colif[:], in_=coli[:])
                nc.sync.dma_start(out=feat_d[5:6, q * 2048:(q + 1) * 2048], in_=colif[:])

            base_reg = cid * BLK
            CH = 2048
            for q in range(4):
                # column-broadcast feature slices [1, CH]
                colf = []
                for k in range(6):
                    ct = wpool.tile([P, CH], f32, tag=f"col{k}")
                    nc.sync.dma_start(out=ct[0:1, :], in_=feat_d[k:k + 1, q * CH:(q + 1) * CH])
                    nc.gpsimd.partition_broadcast(ct[:], ct[0:1, :])
                    colf.append(ct)
                cx1b, cy1b, cx2b, cy2b, careab, colib = colf
                for t in range(NT):
                    start = base_reg + t * P
                    rx1 = wpool.tile([P, 1], f32, tag="rx1")
                    ry1 = wpool.tile([P, 1], f32, tag="ry1")
                    rx2 = wpool.tile([P, 1], f32, tag="rx2")
                    ry2 = wpool.tile([P, 1], f32, tag="ry2")
                    rar = wpool.tile([P, 1], f32, tag="rar")
                    ridx = wpool.tile([P, 1], f32, tag="ridx")
                    for k, rt in enumerate((rx1, ry1, rx2, ry2, rar, ridx)):
                        nc.sync.dma_start(out=rt[:], in_=feat_d[k, bass.ds(start, P), None])

                    e1 = epool.tile([P, CH], f32, tag="e1")
                    e2 = epool.tile([P, CH], f32, tag="e2")
                    e3 = epool.tile([P, CH], f32, tag="e3")
                    e4 = epool.tile([P, CH], f32, tag="e4")
                    e5 = epool.tile([P, CH], f32, tag="e5")
                    # x-chain on DVE
                    nc.vector.tensor_scalar(out=e1[:], in0=cx1b[:], scalar1=rx1[:], scalar2=None, op0=Alu.max)
                    nc.vector.scalar_tensor_tensor(out=e2[:], in0=cx2b[:], scalar=rx2[:], in1=e1[:], op0=Alu.min, op1=Alu.subtract)
                    nc.vector.tensor_scalar(out=e2[:], in0=e2[:], scalar1=0.0, scalar2=None, op0=Alu.max)
                    # y-chain
                    nc.vector.tensor_scalar(out=e4[:], in0=cy1b[:], scalar1=ry1[:], scalar2=None, op0=Alu.max)
                    nc.vector.scalar_tensor_tensor(out=e3[:], in0=cy2b[:], scalar=ry2[:], in1=e4[:], op0=Alu.min, op1=Alu.subtract)
                    nc.vector.tensor_scalar(out=e3[:], in0=e3[:], scalar1=0.0, scalar2=None, op0=Alu.max)
                    # union pre-add + diag stay on DVE (AP scalars unsupported on Pool)
                    nc.vector.tensor_scalar(out=e5[:], in0=careab[:], scalar1=rar[:], scalar2=None, op0=Alu.add)
                    nc.vector.tensor_scalar(out=e4[:], in0=colib[:], scalar1=ridx[:], scalar2=None, op0=Alu.not_equal)
                    # inter, union, edge on DVE
                    nc.vector.tensor_tensor(out=e2[:], in0=e2[:], in1=e3[:], op=Alu.mult)
                    nc.vector.tensor_tensor(out=e1[:], in0=e5[:], in1=e2[:], op=Alu.subtract)
                    nc.vector.scalar_tensor_tensor(out=e1[:], in0=e1[:], scalar=float(IOU_THR), in1=e2[:], op0=Alu.mult, op1=Alu.is_lt)
                    eo = epool.tile([P, CH], mybir.dt.uint8, tag="eo")
                    nc.vector.tensor_tensor(out=eo[:], in0=e1[:], in1=e4[:], op=Alu.mult)
                    nc.sync.dma_start(out=edges[t * P:(t + 1) * P, q * CH:(q + 1) * CH], in_=eo[:])

    nc.compile()
    return nc


def _get_compiled():
    global _compiled
    if _compiled is None:
        _compiled = _build()
    return _compiled


def _host_sweep_and_mask(edge_sym: np.ndarray, conf: np.ndarray) -> np.ndarray:
    """Faithful replication of the reference's sequential sweep + mask build,
    operating on the symmetric edge matrix produced by the device."""
    upper = np.triu(edge_sym, 1).astype(bool)
    a = np.arange(N, dtype=np.int64)
    rows, cols = np.nonzero(upper)
    # uniform per-edge op in lexicographic order:  t = min(a[i],a[j]); a[i]=a[j]=t
    order_rows = rows  # already lex sorted by nonzero
    for i, j in zip(order_rows.tolist(), cols.tolist()):
        ai = a[i]; aj = a[j]
        t = ai if ai < aj else aj
        a[i] = t; a[j] = t
    labels = a
    conf = conf.astype(np.float32)
    cnt = np.zeros(N, np.int64)
    np.add.at(cnt, labels, 1)
    mc = np.full(N, -np.inf, np.float32)
    np.maximum.at(mc, labels, conf)
    cand_g = np.where(conf == mc[labels], np.arange(N), N)
    g = np.full(N, N, np.int64)
    np.minimum.at(g, labels, cand_g)
    gl = g[labels]
    lt = (np.arange(N) < gl).astype(np.int64)
    posr = np.zeros(N, np.int64)
    np.add.at(posr, labels, lt)
    mask = np.zeros(N, bool)
    mask |= (cnt[labels] == 1)
    multi = cnt >= 2
    mask[np.clip(posr[multi], 0, N - 1)] = True
    return mask


def kernel(bboxes_cxcywh: np.ndarray, conf: np.ndarray) -> np.ndarray:
    global last_exec_ns
    from concourse.bass_utils import run_bass_kernel_spmd

    nc = _get_compiled()
    bb = np.ascontiguousarray(bboxes_cxcywh, dtype=np.float32)
    in_maps = [{"bboxes": bb} for _ in range(8)]
    trace = bool(int(os.environ.get("KERNEL_TRACE", "0")))
    res = run_bass_kernel_spmd(nc, in_maps, list(range(8)), trace=trace)
    last_exec_ns = res.exec_time_ns
    edge_sym = np.concatenate([res.results[c]["edges"] for c in range(8)], axis=0)
    return _host_sweep_and_mask(edge_sym, np.asarray(conf))
